# revision 10
# baseline (speedup 1.0000x reference)
"""CrossCoderDecoder forward on 8 trn2 NeuronCores.

x[b,l,d] = sum_f f[b,f] * weight[l,f,d] + bias[l,d]
B=32, L=2, F=65536, D=768, fp32.

Sharding: the F (dict) axis is split 8 ways (8192 features per core).
Each core computes its partial [L, B, D] sums; the host sums the 8
partials and adds the bias (the "all-reduce" of the sharding hint,
done host-side since the output is tiny).

Precision/perf scheme: the kernel is HBM-bound on streaming the weight
(L*FS*D elements/core, each used once), so bytes/element is the whole
game. Both f and weight are cast to SINGLE bf16 (2 B/elem vs fp32's
4): one streaming pass on the PE at 1 col/cyc, fp32 PSUM accumulate.
Total error ~2e-3 max-rel vs the 2e-2 gate.

Weight DMA layout: per chunk of R k-rows ONE dma_start moves a
contiguous [P, R/P, L, D] block (both l interleaved per k-row) into
SBUF. Chunk sizes taper (7x1024, 512, 256, 128, 128 rows: 3.1 MB bulk
transfers for DMA efficiency, small trailing chunks so the
end-of-stream completion latency + final matmul burst expose as
little as possible). All four PSUM accumulators stay open across the
whole kernel; the weight DMAs alternate between the two HWDGE rings
(sync/scalar). The final PSUM drains split across the vector+scalar
engines and the two output DMAs ride the by-then-empty sync/scalar
rings. A single SBUF tile pool with per-tag buffer groups keeps the
Tile semaphore/barrier overhead (prologue+epilogue) down.

Host-side prep packs the weight into the exact SBUF images
(k = kofs(chunk) + p*(R/P) + o at image[p, o]) and permutes f into
fhl[p, j, b] with the matching k order, so the contraction stays
consistent.
"""

import contextlib

import numpy as np
import ml_dtypes

import concourse.bass as bass
import concourse.tile as tile
from concourse import bacc, mybir
from concourse import bass_utils

B, L, F, D = 32, 2, 65536, 768
NCORES = 8
FS = F // NCORES          # 8192 features per core
P = 128
CHUNKS = (1024, 1024, 1024, 1024, 1024, 1024, 1024, 512, 256, 128, 128)
W_BUFS = {8: 5, 4: 1, 2: 1, 1: 2}                 # per-size-class bufs
NSPLITS = ((0, 512), (512, 768))  # PSUM-bank splits of D

assert sum(CHUNKS) == FS
_KOS = [r // P for r in CHUNKS]                   # k-subtiles per chunk
_NJ = sum(_KOS)                                   # 64 subtiles
_CLASSES = sorted(set(_KOS), reverse=True)        # distinct chunk sizes

_F32 = mybir.dt.float32
_BF16 = mybir.dt.bfloat16
_BF16_NP = ml_dtypes.bfloat16

_cache = {}


def set_chunks(chunks: tuple, w_bufs: dict | None = None):
    """Adjust chunking (for tuning sweeps); drops the cached program."""
    global CHUNKS, _KOS, _NJ, _CLASSES
    CHUNKS = tuple(chunks)
    assert sum(CHUNKS) == FS
    _KOS = [r // P for r in CHUNKS]
    _NJ = sum(_KOS)
    _CLASSES = sorted(set(_KOS), reverse=True)
    if w_bufs is not None:
        W_BUFS.update(w_bufs)
    _cache.clear()


def _build():
    """Build + schedule the (per-core identical) Bass program once."""
    nc = bacc.Bacc("TRN2", target_bir_lowering=False, debug=False)

    fhl = nc.dram_tensor("fhl", [P, _NJ, B], _BF16, kind="ExternalInput").ap()
    wdram = {
        ko: nc.dram_tensor(
            f"w{ko}", [_KOS.count(ko), P, ko, L, D], _BF16, kind="ExternalInput"
        ).ap()
        for ko in _CLASSES
    }
    out = nc.dram_tensor("out", [L, B, D], _F32, kind="ExternalOutput").ap()

    with tile.TileContext(nc) as tc:
        with (
            tc.tile_pool(name="sb", bufs=1) as sb,
            tc.tile_pool(name="psum", bufs=1, space="PSUM") as psum,
        ):
            # fhl rides the SP HWDGE ring FIRST: the ACT ring has been
            # observed to start ~3us late and ramp slowly, which (with f
            # on it) starved the PE's first stationary load for ~12us.
            # The SP ring's first transfer lands within ~2us.
            f_sb = sb.tile([P, _NJ, B], _BF16, tag="f", bufs=1, name="f_sb")
            nc.sync.dma_start(f_sb[:], fhl[:])

            ps = [
                [
                    psum.tile([B, n1 - n0], _F32, name=f"ps_{l}_{i}")
                    for i, (n0, n1) in enumerate(NSPLITS)
                ]
                for l in range(L)
            ]
            jofs = 0
            cls_idx = {ko: 0 for ko in _CLASSES}
            for ci, r in enumerate(CHUNKS):
                ko = r // P
                wt = sb.tile(
                    [P, ko, L, D], _BF16, tag=f"w{ko}", bufs=W_BUFS[ko],
                    name=f"wt{ci}",
                )
                dma_eng = nc.scalar if ci % 2 == 0 else nc.sync
                dma_eng.dma_start(wt[:], wdram[ko][cls_idx[ko]])
                for o in range(ko):
                    j = jofs + o
                    for l in range(L):
                        for i, (n0, n1) in enumerate(NSPLITS):
                            nc.tensor.matmul(
                                ps[l][i][:],
                                f_sb[:, j, :],
                                wt[:, o, l, n0:n1],
                                start=(j == 0),
                                stop=(j == _NJ - 1),
                            )
                cls_idx[ko] += 1
                jofs += ko
            # Drain: copies split across vector+scalar, out DMAs on the
            # two HWDGE rings (empty once the last w chunk is queued).
            for l in range(L):
                out_sb = sb.tile([B, D], _F32, tag="o", bufs=2, name=f"o{l}")
                nc.vector.tensor_copy(out=out_sb[:, 0:512], in_=ps[l][0][:])
                nc.scalar.copy(out=out_sb[:, 512:768], in_=ps[l][1][:])
                (nc.sync if l == 0 else nc.scalar).dma_start(out[l], out_sb[:])

    nc.compile()
    return nc


def _prep_core(f_core: np.ndarray, w_core: np.ndarray) -> dict:
    """Build the per-core input map.

    f_core [B, FS] fp32 -> fhl [P, NJ, B] bf16 with
    fhl[p, jofs+o, b] = f[b, kofs + p*ko + o] per chunk.
    w_core [L, FS, D] fp32 -> one [cnt, P, ko, L, D] bf16 image per
    chunk-size class, matching the kernel's DMA order.
    """
    fh = f_core.astype(_BF16_NP)
    wh = w_core.astype(_BF16_NP)          # [L, FS, D]
    whT = np.ascontiguousarray(wh.transpose(1, 0, 2))  # [FS, L, D]
    fhl = np.empty((P, _NJ, B), dtype=_BF16_NP)
    wimgs = {ko: [] for ko in _CLASSES}
    kofs = 0
    jofs = 0
    for r in CHUNKS:
        ko = r // P
        # k = kofs + p*ko + o  (C-order reshape)
        fhl[:, jofs : jofs + ko, :] = (
            fh[:, kofs : kofs + r].T.reshape(P, ko, B)
        )
        wimgs[ko].append(whT[kofs : kofs + r].reshape(P, ko, L, D))
        kofs += r
        jofs += ko
    in_map = {"fhl": np.ascontiguousarray(fhl)}
    for ko in _CLASSES:
        in_map[f"w{ko}"] = np.ascontiguousarray(np.stack(wimgs[ko]))
    return in_map


def kernel(f: np.ndarray, weight: np.ndarray, bias: np.ndarray) -> np.ndarray:
    f = np.asarray(f, dtype=np.float32)
    weight = np.asarray(weight, dtype=np.float32)
    bias = np.asarray(bias, dtype=np.float32)

    if "nc" not in _cache:
        _cache["nc"] = _build()
    nc = _cache["nc"]

    in_maps = []
    for c in range(NCORES):
        sl = slice(c * FS, (c + 1) * FS)
        in_maps.append(_prep_core(f[:, sl], weight[:, sl, :]))

    res = bass_utils.run_bass_kernel_spmd(nc, in_maps, core_ids=list(range(NCORES)))
    partial = np.stack([r["out"] for r in res.results])  # [NCORES, L, B, D]
    total = partial.sum(axis=0)                          # [L, B, D]
    x = total.transpose(1, 0, 2) + bias[None, :, :]      # [B, L, D]
    return x.astype(np.float32)
